# revision 20
# baseline (speedup 1.0000x reference)
"""BiMamba Trainium2 kernel (v2).

Sharding: each of the 8 cores owns a 256-channel slice of d_inner for BOTH
directions (fwd+rev share in_proj/out_proj, so the reversed direction's
in_proj output is just a flipped view of the forward one).

v2 changes vs v1:
  - x-half of in_proj runs first so conv/x_proj/AllReduce start earlier;
    z-half + silu gating overlap the AllReduce.
  - AllReduce payload in bf16 (halves collective time).
  - dt/x_dbl pipeline in bf16; ACT exp/ln calls batched per function to
    avoid ACT table thrash.
  - Scan-phase reduction over states uses two accumulator chains per
    (dir, ptile): one on DVE, one on GpSimd, merged at the end. Keeps
    both engines ~equally loaded (DVE also owns the 64 scans + hC mults,
    GpSimd owns the dbx mults).
  - full-L hC tiles (no 1024-chunking).
  - out_proj ReduceScatter split in 2 overlapped chunks (host reassembles
    the permuted row blocks).
"""

import os
import sys

sys.path.insert(0, "/opt/trn_rl_repo")

import numpy as np
import ml_dtypes

# ---------------------------------------------------------------- constants
P = 128           # partitions
L = 2048          # sequence length
DM = 1024         # d_model
DI = 2048         # d_inner
NST = 16          # d_state
RK = 64           # dt_rank
KCONV = 4         # conv width
NCORES = 8
CH = DI // NCORES          # channels per core per direction = 256
NPT = CH // P              # channel ptiles per core = 2
FB = 512                   # matmul moving free chunk
NFB = L // FB              # 4
PAD = KCONV - 1            # causal pad = 3
NXP = RK + 2 * NST         # 96
NRS = 4                    # ReduceScatter chunks (token blocks)
RSROWS = L // NRS          # rows per RS chunk = 1024


def build_program(num_cores=NCORES, skip_scan=False):
    """Build the SPMD Bass program (same NEFF on every core)."""
    import concourse.bass as bass
    import concourse.mybir as mybir
    import concourse.tile as tile
    from concourse import bacc
    from contextlib import ExitStack

    dt = mybir.dt
    AF = mybir.ActivationFunctionType
    OP = mybir.AluOpType

    nc = bacc.Bacc(
        "TRN2",
        target_bir_lowering=False,
        debug=False,
        enable_asserts=False,
        num_devices=num_cores,
    )

    # ------------------------------------------------------------- dram I/O
    hidden = nc.dram_tensor("hidden", [L, DM], dt.float32, kind="ExternalInput")
    w_inT = nc.dram_tensor("w_inT", [DM, 2 * CH], dt.bfloat16, kind="ExternalInput")
    w_outT = nc.dram_tensor("w_outT", [CH, DM], dt.bfloat16, kind="ExternalInput")
    w_xT = {}
    w_dtT = {}
    conv_w = {}
    conv_b = {}
    dt_b = {}
    A_in = {}
    D_in = {}
    for d in ("f", "r"):
        w_xT[d] = nc.dram_tensor(f"w_xT_{d}", [CH, NXP], dt.bfloat16,
                                 kind="ExternalInput")
        w_dtT[d] = nc.dram_tensor(f"w_dtT_{d}", [RK, CH], dt.bfloat16,
                                  kind="ExternalInput")
        conv_w[d] = nc.dram_tensor(f"conv_w_{d}", [CH, KCONV], dt.float32,
                                   kind="ExternalInput")
        conv_b[d] = nc.dram_tensor(f"conv_b_{d}", [CH, 1], dt.float32,
                                   kind="ExternalInput")
        dt_b[d] = nc.dram_tensor(f"dt_b_{d}", [CH, 1], dt.float32,
                                 kind="ExternalInput")
        A_in[d] = nc.dram_tensor(f"A_{d}", [CH, NST], dt.float32,
                                 kind="ExternalInput")
        D_in[d] = nc.dram_tensor(f"D_{d}", [CH, 1], dt.float32,
                                 kind="ExternalInput")
    ident = nc.dram_tensor("ident", [P, P], dt.float32, kind="ExternalInput")
    out = nc.dram_tensor("out", [L // num_cores, DM], dt.float32,
                         kind="ExternalOutput")

    NKB = DM // P  # 8
    NTT = L // P   # 16
    WPAD = L + 2 * PAD
    TW = 2 * L     # pb-fused tile width (pb0 cols 0:L, pb1 cols L:TW)

    with tile.TileContext(nc) as tc:
        ctx = ExitStack()
        with ctx:
            dram = ctx.enter_context(tc.tile_pool(name="dram", bufs=1, space="DRAM"))
            consts = ctx.enter_context(tc.tile_pool(name="consts", bufs=1))
            psum_mm = ctx.enter_context(
                tc.tile_pool(name="psum_mm", bufs=3, space="PSUM"))

            ident_sb = consts.tile([P, P], dt.float32)
            nc.sync.dma_start(ident_sb[:], ident[:])

            # ---------------- persistent pools (live through the scan phase)
            gz_pool = ctx.enter_context(tc.tile_pool(name="gzp", bufs=1))
            gz = {}
            for d in ("f", "r"):
                gz[d] = gz_pool.tile([P, TW], dt.bfloat16, name=f"gz{d}",
                                     tag=f"gz{d}")
            yc_pool = ctx.enter_context(tc.tile_pool(name="ycp", bufs=2))
            oev_pool = ctx.enter_context(tc.tile_pool(name="oevp", bufs=1))
            dt_pool = ctx.enter_context(tc.tile_pool(name="dtp", bufs=1))
            dtx_pool = ctx.enter_context(tc.tile_pool(name="dtxp", bufs=2))
            xc_pool = ctx.enter_context(tc.tile_pool(name="xcp", bufs=2))
            et_pool = ctx.enter_context(tc.tile_pool(name="etp", bufs=4))
            xdbl_pool = ctx.enter_context(tc.tile_pool(name="xdblp", bufs=1))

            xdbl_part = {}
            xdbl_sum = {}
            for d in ("f", "r"):
                xdbl_part[d] = dram.tile([NXP, L], dt.bfloat16, name=f"xp{d}",
                                         tag=f"xp{d}")
                xdbl_sum[d] = dram.tile([NXP, L], dt.bfloat16,
                                        addr_space="Shared", name=f"xs{d}",
                                        tag=f"xs{d}")
            pout = dram.tile([L, DM], dt.float32)
            pout_rs = dram.tile([L // num_cores, DM], dt.float32)

            # stage-limited pools
            ctxB = ExitStack()
            xpad_pool = ctxB.enter_context(tc.tile_pool(name="xpadp", bufs=1))
            cacc_pool = ctxB.enter_context(tc.tile_pool(name="caccp", bufs=2))
            xev_pool = ctxB.enter_context(tc.tile_pool(name="xevp", bufs=2))
            ctxA = ExitStack()
            hT_pool = ctxA.enter_context(tc.tile_pool(name="hTp", bufs=1))
            hnat_pool = ctxA.enter_context(tc.tile_pool(name="hnatp", bufs=4))
            w_in_pool = ctxA.enter_context(tc.tile_pool(name="winp", bufs=1))

            # ------------------------------------------- stage 1: hT = hidden^T
            psum_tp = ctxA.enter_context(
                tc.tile_pool(name="psum_tp", bufs=3, space="PSUM"))
            hT = {(k, q): hT_pool.tile([P, 4 * P], dt.bfloat16,
                                       name=f"hT{k}_{q}", tag=f"hT{k}_{q}")
                  for k in range(NKB) for q in range(NTT // 4)}
            for q in range(NTT // 4):
                hn = []
                for j in range(4):
                    t = hnat_pool.tile([P, DM], dt.float32, name="hnat", tag="hnat")
                    nc.sync.dma_start(
                        t[:], hidden[(q * 4 + j) * P:(q * 4 + j + 1) * P, :])
                    hn.append(t)
                for kb in range(NKB):
                    pt = psum_tp.tile([P, 4 * P], dt.float32, name="tp", tag="tp")
                    for j in range(4):
                        nc.tensor.transpose(
                            pt[:, j * P:(j + 1) * P],
                            hn[j][:, kb * P:(kb + 1) * P],
                            ident_sb[:],
                        )
                    nc.scalar.copy(hT[kb, q][:], pt[:])

            # ------------------------------------------- stage 2: in_proj x-half
            w_in_sb = [w_in_pool.tile([P, 2 * CH], dt.bfloat16, name=f"win{k}",
                                      tag=f"win{k}") for k in range(NKB)]
            for k in range(NKB):
                nc.sync.dma_start(w_in_sb[k][:], w_inT[k * P:(k + 1) * P, :])

            # const loads (after the latency-critical hidden/w_in DMAs)
            conv_w_sb = {}
            conv_b_sb = {}
            dt_b_sb = {}
            A_sb = {}
            D_sb = {}
            for d in ("f", "r"):
                for pb in range(NPT):
                    ps = slice(pb * P, (pb + 1) * P)
                    for nm, store, srct, shape in (
                        ("cw", conv_w_sb, conv_w, [P, KCONV]),
                        ("cb", conv_b_sb, conv_b, [P, 1]),
                        ("db", dt_b_sb, dt_b, [P, 1]),
                        ("A", A_sb, A_in, [P, NST]),
                        ("Dc", D_sb, D_in, [P, 1]),
                    ):
                        t = consts.tile(shape, dt.float32, name=f"{nm}{d}{pb}",
                                        tag=f"{nm}{d}{pb}")
                        nc.sync.dma_start(t[:], srct[d][ps, :])
                        store[d, pb] = t
            w_dt_sb = {}
            for d in ("f", "r"):
                w_dt_sb[d] = consts.tile([RK, CH], dt.bfloat16, name=f"wdt{d}",
                                         tag=f"wdt{d}")
                nc.sync.dma_start(w_dt_sb[d][:], w_dtT[d][:])
            w_x_sb = {}
            for d in ("f", "r"):
                for pb in range(NPT):
                    t = consts.tile([P, NXP], dt.bfloat16, name=f"wx{d}{pb}",
                                    tag=f"wx{d}{pb}")
                    nc.sync.dma_start(t[:], w_xT[d][pb * P:(pb + 1) * P, :])
                    w_x_sb[d, pb] = t
            w_out_sb = []
            for pb in range(NPT):
                t = consts.tile([P, DM], dt.bfloat16, name=f"wo{pb}", tag=f"wo{pb}")
                nc.sync.dma_start(t[:], w_outT[pb * P:(pb + 1) * P, :])
                w_out_sb.append(t)

            xpad = [xpad_pool.tile([P, WPAD], dt.bfloat16, name=f"xpad{pb}",
                                   tag=f"xpad{pb}") for pb in range(NPT)]
            for pb in range(NPT):
                nc.vector.memset(xpad[pb][:, 0:PAD], 0.0)
                nc.vector.memset(xpad[pb][:, PAD + L:WPAD], 0.0)

            for fb in range(NFB):
                for mb in range(NPT):
                    pm = psum_mm.tile([P, FB], dt.float32, name="mm", tag="mm")
                    for k in range(NKB):
                        nc.tensor.matmul(
                            pm[:],
                            w_in_sb[k][:, mb * P:(mb + 1) * P],
                            hT[k, fb][:],
                            start=(k == 0),
                            stop=(k == NKB - 1),
                        )
                    nc.scalar.copy(
                        xpad[mb][:, PAD + fb * FB: PAD + (fb + 1) * FB], pm[:])

            xc = {}

            def conv_block(d):
                """causal depthwise conv + silu -> fused xc[d] [P, TW]."""
                xc[d] = xc_pool.tile([P, TW], dt.bfloat16, name=f"xc{d}",
                                     tag="xc")
                for pb in range(NPT):
                    cw = conv_w_sb[d, pb]
                    cb = conv_b_sb[d, pb]
                    if d == "f":
                        taps = [xpad[pb][:, k:k + L] for k in range(KCONV)]
                    else:
                        taps = [xpad[pb][:, 2 * PAD - k: 2 * PAD - k + L][:, ::-1]
                                for k in range(KCONV)]
                    acc = cacc_pool.tile([P, L], dt.bfloat16, name="cacc", tag="cacc")
                    nc.scalar.activation(acc[:], taps[0], AF.Identity,
                                         bias=cb[:, 0:1], scale=cw[:, 0:1])
                    for k in range(1, KCONV):
                        acc2 = cacc_pool.tile([P, L], dt.bfloat16, name="cacc",
                                              tag="cacc")
                        nc.vector.scalar_tensor_tensor(
                            acc2[:], taps[k], cw[:, k:k + 1], acc[:],
                            OP.mult, OP.add)
                        acc = acc2
                    nc.scalar.activation(
                        xc[d][:, pb * L:(pb + 1) * L], acc[:], AF.Silu)

            def xproj_block(d):
                for pb in range(NPT):
                    for fb in range(NFB):
                        pm = psum_mm.tile([NXP, FB], dt.float32, name="mmx",
                                          tag="mm")
                        nc.tensor.matmul(
                            pm[:],
                            w_x_sb[d, 0][:],
                            xc[d][:, fb * FB:(fb + 1) * FB] if pb == 0 else
                            xc[d][:, L + fb * FB:L + (fb + 1) * FB],
                            start=True, stop=False,
                            skip_group_check=True,
                        ) if False else None
                # standard: accumulate the two pb contributions per fb chunk
                for fb in range(NFB):
                    pm = psum_mm.tile([NXP, FB], dt.float32, name="mmx", tag="mm")
                    for pb in range(NPT):
                        nc.tensor.matmul(
                            pm[:],
                            w_x_sb[d, pb][:],
                            xc[d][:, pb * L + fb * FB: pb * L + (fb + 1) * FB],
                            start=(pb == 0),
                            stop=(pb == NPT - 1),
                        )
                    xev = xev_pool.tile([NXP, FB], dt.bfloat16, name="xev",
                                        tag="xev")
                    nc.scalar.copy(xev[:], pm[:])
                    nc.sync.dma_start(
                        xdbl_part[d][:, fb * FB:(fb + 1) * FB], xev[:])

            def allreduce(d):
                nc.gpsimd.collective_compute(
                    "AllReduce",
                    OP.add,
                    replica_groups=[list(range(num_cores))],
                    ins=[xdbl_part[d][:].opt()],
                    outs=[xdbl_sum[d][:].opt()],
                )

            conv_block("f")
            xproj_block("f")
            allreduce("f")
            conv_block("r")
            xproj_block("r")
            allreduce("r")

            # ------------------- z half of in_proj + silu gating (overlaps AR)
            for mb in range(NPT, 2 * NPT):
                for fb in range(NFB):
                    pm = psum_mm.tile([P, FB], dt.float32, name="mm", tag="mm")
                    for k in range(NKB):
                        nc.tensor.matmul(
                            pm[:],
                            w_in_sb[k][:, mb * P:(mb + 1) * P],
                            hT[k, fb][:],
                            start=(k == 0),
                            stop=(k == NKB - 1),
                        )
                    pb = mb - NPT
                    nc.scalar.activation(
                        gz["f"][:, pb * L + fb * FB: pb * L + (fb + 1) * FB],
                        pm[:], AF.Silu)
                    grev = gz["r"][:, pb * L:(pb + 1) * L][:, ::-1]
                    nc.scalar.activation(
                        grev[:, fb * FB:(fb + 1) * FB], pm[:], AF.Silu)

            ctxA.close()

            dt_sb = {}
            dtx = {}
            xdbl = {}
            y = {}
            accv = {}

            def dt_block(d, xc_tile):
                """x_dbl dt-rows -> softplus dt, dtx, chain seed (D*x)."""
                xdbl[d] = xdbl_pool.tile([RK, L], dt.bfloat16, name="xdbl",
                                         tag="xdbl")
                nc.sync.dma_start(xdbl[d][:], xdbl_sum[d][0:RK, :])
                dt_sb[d] = dt_pool.tile([P, TW], dt.bfloat16, name="dtt",
                                        tag=f"dtt{d}")
                for pb in range(NPT):
                    ets = {}
                    for fb in range(NFB):
                        pm = psum_mm.tile([P, FB], dt.float32, name="mm",
                                          tag="mm")
                        nc.tensor.matmul(
                            pm[:],
                            w_dt_sb[d][:, pb * P:(pb + 1) * P],
                            xdbl[d][:, fb * FB:(fb + 1) * FB],
                            start=True, stop=True)
                        et = et_pool.tile([P, FB], dt.bfloat16, name="etmp",
                                          tag="etmp")
                        nc.scalar.activation(
                            et[:], pm[:], AF.Exp, bias=dt_b_sb[d, pb][:, 0:1])
                        ets[fb] = et
                    for fb in range(NFB):
                        nc.scalar.activation(
                            dt_sb[d][:, pb * L + fb * FB: pb * L + (fb + 1) * FB],
                            ets[fb][:], AF.Ln, bias=1.0)
                tx = dtx_pool.tile([P, TW], dt.bfloat16, name="dtx", tag="dtx")
                nc.vector.tensor_mul(tx[:], dt_sb[d][:], xc_tile[:])
                dtx[d] = tx
                # poison the pb seam so the fused scan restarts at col L
                nc.vector.memset(dt_sb[d][:, L:L + 1], 1.0e9)

            def seed_chain(d, xc_tile):
                """accv[d] = D*x (the reduction chain grows from this seed)."""
                s = accv_pool.tile([P, TW], dt.bfloat16, name="seed", tag="acc")
                for pb in range(NPT):
                    nc.vector.tensor_scalar_mul(
                        s[:, pb * L:(pb + 1) * L],
                        xc_tile[:, pb * L:(pb + 1) * L],
                        D_sb[d, pb][:, 0:1])
                accv[d] = s

            def scan_iter(d, n):
                rb = xdbl_sum[d][RK + n: RK + n + 1, :]
                rc = xdbl_sum[d][RK + NST + n: RK + NST + n + 1, :]
                bb = bbc_pool.tile([P, TW], dt.bfloat16, name="bbc", tag="bbc")
                nc.sync.dma_start(
                    bb[:], bass.AP(rb.tensor, rb.offset, [[0, P], [0, 2], [1, L]]))
                cbt = cbc_pool.tile([P, TW], dt.bfloat16, name="cbc", tag="cbc")
                nc.sync.dma_start(
                    cbt[:], bass.AP(rc.tensor, rc.offset, [[0, P], [0, 2], [1, L]]))
                da = da_pool.tile([P, TW], dt.float16, name="da", tag="da")
                for pb in range(NPT):
                    nc.scalar.activation(
                        da[:, pb * L:(pb + 1) * L],
                        dt_sb[d][:, pb * L:(pb + 1) * L], AF.Exp,
                        scale=A_sb[d, pb][:, n:n + 1])
                dbx = dbx_pool.tile([P, TW], dt.bfloat16, name="dbx", tag="dbx")
                nc.gpsimd.tensor_tensor(dbx[:], dtx[d][:], bb[:], OP.mult)
                h = h_pool.tile([P, TW], dt.bfloat16, name="h", tag="h")
                nc.vector.tensor_tensor_scan(
                    h[:], da[:], dbx[:], 0.0, OP.mult, OP.add)
                hc = hc_pool.tile([P, TW], dt.bfloat16, name="hc", tag="hc")
                nc.vector.tensor_mul(hc[:], h[:], cbt[:])
                s = accv_pool.tile([P, TW], dt.bfloat16, name="acc", tag="acc")
                nc.vector.tensor_add(s[:], accv[d][:], hc[:])
                accv[d] = s

            def collapse(d):
                yt = y_pool.tile([P, TW], dt.bfloat16, name="y", tag=f"y{d}")
                nc.vector.tensor_mul(yt[:], accv[d][:], gz[d][:])
                y[d] = yt

            dt_block("f", xc["f"])
            ctxB.close()

            # ------------------------------------------- scan pools
            bbc_pool = ctx.enter_context(tc.tile_pool(name="bbcp", bufs=2))
            cbc_pool = ctx.enter_context(tc.tile_pool(name="cbcp", bufs=2))
            da_pool = ctx.enter_context(tc.tile_pool(name="dap", bufs=2))
            dbx_pool = ctx.enter_context(tc.tile_pool(name="dbxp", bufs=2))
            h_pool = ctx.enter_context(tc.tile_pool(name="hp", bufs=2))
            hc_pool = ctx.enter_context(tc.tile_pool(name="hcp", bufs=1))
            accv_pool = ctx.enter_context(tc.tile_pool(name="accvp", bufs=2))
            y_pool = ctx.enter_context(tc.tile_pool(name="yp", bufs=1))

            # (seed uses accv_pool which must exist before dt_block refs run;
            # Python late-binding makes this fine because dt_block("f") above
            # only used pools already created. seed_chain for "f" runs here.)
            seed_chain("f", xc["f"])

            if skip_scan:
                for d in ("f", "r"):
                    if d == "r":
                        dt_block("r", xc["r"])
                    yt = y_pool.tile([P, TW], dt.bfloat16, name="y", tag=f"y{d}")
                    nc.vector.tensor_mul(yt[:], dtx[d][:], gz[d][:])
                    y[d] = yt
            else:
                for n in range(6):
                    scan_iter("f", n)
                # dt_block("r") lands here (AR_r done by now)
                dt_block("r", xc["r"])
                for n in range(6, NST):
                    scan_iter("f", n)
                collapse("f")
                seed_chain("r", xc["r"])
                for n in range(NST):
                    scan_iter("r", n)
                collapse("r")

            # ------------------------------------------- out_proj + RS
            for rs in range(NRS):
                for tbo in range(RSROWS // P):
                    tb = rs * (RSROWS // P) + tbo
                    ycb = {}
                    for pb in range(NPT):
                        yct = yc_pool.tile([P, P], dt.bfloat16, name="ycb",
                                           tag="ycb")
                        nc.vector.tensor_add(
                            yct[:],
                            y["f"][:, pb * L + tb * P: pb * L + (tb + 1) * P],
                            y["r"][:, pb * L:(pb + 1) * L][:, ::-1]
                            [:, tb * P:(tb + 1) * P])
                        ycb[pb] = yct
                    for fb in range(DM // FB):
                        pm = psum_mm.tile([P, FB], dt.float32, name="mm",
                                          tag="mm")
                        for pb in range(NPT):
                            nc.tensor.matmul(
                                pm[:],
                                ycb[pb][:],
                                w_out_sb[pb][:, fb * FB:(fb + 1) * FB],
                                start=(pb == 0),
                                stop=(pb == NPT - 1),
                            )
                        oev = oev_pool.tile([P, FB], dt.float32, name="oev",
                                            tag="oev")
                        nc.scalar.copy(oev[:], pm[:])
                        nc.sync.dma_start(
                            pout[tb * P:(tb + 1) * P, fb * FB:(fb + 1) * FB],
                            oev[:])
                nc.gpsimd.collective_compute(
                    "ReduceScatter",
                    OP.add,
                    replica_groups=[list(range(num_cores))],
                    ins=[pout[rs * RSROWS:(rs + 1) * RSROWS, :].opt()],
                    outs=[pout_rs[rs * (RSROWS // num_cores):
                                  (rs + 1) * (RSROWS // num_cores), :].opt()],
                )
                nc.sync.dma_start(
                    out[rs * (RSROWS // num_cores):
                        (rs + 1) * (RSROWS // num_cores), :],
                    pout_rs[rs * (RSROWS // num_cores):
                            (rs + 1) * (RSROWS // num_cores), :])

    return nc


# ---------------------------------------------------------------- host side
def _make_in_maps(inputs):
    """Slice/transpose the full inputs into per-core input dicts."""
    h = np.ascontiguousarray(np.asarray(inputs["hidden_states"],
                                        dtype=np.float32).reshape(L, DM))
    w_in = np.asarray(inputs["in_proj_w"], dtype=np.float32)     # (2DI, DM)
    w_out = np.asarray(inputs["out_proj_w"], dtype=np.float32)   # (DM, DI)
    ident = np.eye(P, dtype=np.float32)

    in_maps = []
    for c in range(NCORES):
        sl = slice(c * CH, (c + 1) * CH)
        m = {"hidden": h, "ident": ident}
        w_slice = np.concatenate(
            [w_in[sl, :], w_in[DI + c * CH: DI + (c + 1) * CH, :]], axis=0)
        m["w_inT"] = np.ascontiguousarray(
            w_slice.T).astype(ml_dtypes.bfloat16)                 # (DM, 2CH)
        m["w_outT"] = np.ascontiguousarray(
            w_out[:, sl].T).astype(ml_dtypes.bfloat16)            # (CH, DM)
        for d, tag in (("f", "_f"), ("r", "_r")):
            w_x = np.asarray(inputs[f"x_proj_w{tag}"], dtype=np.float32)
            m[f"w_xT_{d}"] = np.ascontiguousarray(
                w_x[:, sl].T).astype(ml_dtypes.bfloat16)          # (CH, 96)
            w_dt = np.asarray(inputs[f"dt_proj_w{tag}"], dtype=np.float32)
            m[f"w_dtT_{d}"] = np.ascontiguousarray(
                w_dt[sl, :].T).astype(ml_dtypes.bfloat16)         # (RK, CH)
            m[f"conv_w_{d}"] = np.ascontiguousarray(
                np.asarray(inputs[f"conv_w{tag}"], dtype=np.float32)[sl, :])
            m[f"conv_b_{d}"] = np.ascontiguousarray(
                np.asarray(inputs[f"conv_b{tag}"], dtype=np.float32)[sl, None])
            m[f"dt_b_{d}"] = np.ascontiguousarray(
                np.asarray(inputs[f"dt_proj_b{tag}"], dtype=np.float32)[sl, None])
            m[f"A_{d}"] = np.ascontiguousarray(
                -np.exp(np.asarray(inputs[f"A_log{tag}"], dtype=np.float32)[sl, :]))
            m[f"D_{d}"] = np.ascontiguousarray(
                np.asarray(inputs[f"D{tag}"], dtype=np.float32)[sl, None])
        in_maps.append(m)
    return in_maps


_CACHED = {}


def _install_ntff_hook_shim():
    """The agent image's antenv lacks axon_hooks; provide it and register
    the ctypes-based NTFF profile hook from trn_agent_boot."""
    import types
    try:
        import antenv.axon_hooks  # noqa: F401
        return
    except ImportError:
        pass
    import antenv
    mod = types.ModuleType("antenv.axon_hooks")
    _state = {"h": None}
    mod.get_axon_ntff_profile_hook = lambda: _state["h"]
    mod.set_axon_ntff_profile_hook = lambda h: _state.__setitem__("h", h)
    sys.modules["antenv.axon_hooks"] = mod
    antenv.axon_hooks = mod
    try:
        from trn_agent_boot.trn_boot import _ntff_profile_via_ctypes
        hook = _ntff_profile_via_ctypes("/opt/axon/libaxon_pjrt.so")
        if hook is not None:
            mod.set_axon_ntff_profile_hook(hook)
    except Exception:
        pass


def _install_hook_err_capture():
    """Wrap the neuronx_cc hook so compile errors land in hook_err.log
    instead of being swallowed by the PJRT boundary."""
    import traceback
    import concourse.bass2jax as b2j
    if getattr(b2j, "_err_capture_installed", False):
        return
    orig = b2j.neuronx_cc_hook

    def wrapped(*a):
        try:
            return orig(*a)
        except Exception:
            with open("/tmp/hook_err.log", "w") as f:
                f.write(traceback.format_exc())
            raise

    b2j.neuronx_cc_hook = wrapped
    b2j._err_capture_installed = True


def kernel(**inputs):
    from concourse.bass_utils import run_bass_kernel_spmd

    _install_ntff_hook_shim()
    _install_hook_err_capture()

    if "nc" not in _CACHED:
        from concourse.bass_interp import get_hw_module
        nc = build_program(
            skip_scan=bool(int(os.environ.get("KERNEL_SKIP_SCAN", "0"))))
        nc.finalize()  # bacc: register allocation, library/ACT-table loads
        nc.m = get_hw_module(nc.m)  # strip sim-only callback instructions
        _CACHED["nc"] = nc
    nc = _CACHED["nc"]

    in_maps = _make_in_maps(inputs)
    res = run_bass_kernel_spmd(
        nc, in_maps, core_ids=list(range(NCORES)),
        trace=bool(int(os.environ.get("KERNEL_TRACE", "0"))),
    )
    _CACHED["last_result"] = res
    # Chunked ReduceScatter permutes row ownership: core c's out rows are
    # [rs*RSROWS + c*(RSROWS/8) : +RSROWS/8) for each rs chunk.
    rows = RSROWS // NCORES
    full = np.empty((L, DM), dtype=np.float32)
    for c in range(NCORES):
        o = res.results[c]["out"]
        for rs in range(NRS):
            full[rs * RSROWS + c * rows: rs * RSROWS + (c + 1) * rows, :] = \
                o[rs * rows:(rs + 1) * rows, :]
    return full.reshape(1, L, DM)


if __name__ == "__main__":
    nc = build_program()
    try:
        n = sum(len(bb.instructions) for bb in nc.main_func.blocks)
    except Exception:
        n = "?"
    print("build ok; instructions:", n)


# revision 21
# speedup vs baseline: 1.0451x; 1.0451x over previous
"""BiMamba Trainium2 kernel (v2).

Sharding: each of the 8 cores owns a 256-channel slice of d_inner for BOTH
directions (fwd+rev share in_proj/out_proj, so the reversed direction's
in_proj output is just a flipped view of the forward one).

v2 changes vs v1:
  - x-half of in_proj runs first so conv/x_proj/AllReduce start earlier;
    z-half + silu gating overlap the AllReduce.
  - AllReduce payload in bf16 (halves collective time).
  - dt/x_dbl pipeline in bf16; ACT exp/ln calls batched per function to
    avoid ACT table thrash.
  - Scan-phase reduction over states uses two accumulator chains per
    (dir, ptile): one on DVE, one on GpSimd, merged at the end. Keeps
    both engines ~equally loaded (DVE also owns the 64 scans + hC mults,
    GpSimd owns the dbx mults).
  - full-L hC tiles (no 1024-chunking).
  - out_proj ReduceScatter split in 2 overlapped chunks (host reassembles
    the permuted row blocks).
"""

import os
import sys

sys.path.insert(0, "/opt/trn_rl_repo")

import numpy as np
import ml_dtypes

# ---------------------------------------------------------------- constants
P = 128           # partitions
L = 2048          # sequence length
DM = 1024         # d_model
DI = 2048         # d_inner
NST = 16          # d_state
RK = 64           # dt_rank
KCONV = 4         # conv width
NCORES = 8
CH = DI // NCORES          # channels per core per direction = 256
NPT = CH // P              # channel ptiles per core = 2
FB = 512                   # matmul moving free chunk
NFB = L // FB              # 4
PAD = KCONV - 1            # causal pad = 3
NXP = RK + 2 * NST         # 96
NRS = 4                    # ReduceScatter chunks (token blocks)
RSROWS = L // NRS          # rows per RS chunk = 1024


def build_program(num_cores=NCORES, skip_scan=False):
    """Build the SPMD Bass program (same NEFF on every core)."""
    import concourse.bass as bass
    import concourse.mybir as mybir
    import concourse.tile as tile
    from concourse import bacc
    from contextlib import ExitStack

    dt = mybir.dt
    AF = mybir.ActivationFunctionType
    OP = mybir.AluOpType

    nc = bacc.Bacc(
        "TRN2",
        target_bir_lowering=False,
        debug=False,
        enable_asserts=False,
        num_devices=num_cores,
    )

    # ------------------------------------------------------------- dram I/O
    hidden = nc.dram_tensor("hidden", [L, DM], dt.float32, kind="ExternalInput")
    w_inT = nc.dram_tensor("w_inT", [DM, 2 * CH], dt.bfloat16, kind="ExternalInput")
    w_outT = nc.dram_tensor("w_outT", [CH, DM], dt.bfloat16, kind="ExternalInput")
    w_xT = {}
    w_dtT = {}
    conv_w = {}
    conv_b = {}
    dt_b = {}
    A_in = {}
    D_in = {}
    for d in ("f", "r"):
        w_xT[d] = nc.dram_tensor(f"w_xT_{d}", [CH, NXP], dt.bfloat16,
                                 kind="ExternalInput")
        w_dtT[d] = nc.dram_tensor(f"w_dtT_{d}", [RK, CH], dt.bfloat16,
                                  kind="ExternalInput")
        conv_w[d] = nc.dram_tensor(f"conv_w_{d}", [CH, KCONV], dt.float32,
                                   kind="ExternalInput")
        conv_b[d] = nc.dram_tensor(f"conv_b_{d}", [CH, 1], dt.float32,
                                   kind="ExternalInput")
        dt_b[d] = nc.dram_tensor(f"dt_b_{d}", [CH, 1], dt.float32,
                                 kind="ExternalInput")
        A_in[d] = nc.dram_tensor(f"A_{d}", [CH, NST], dt.float32,
                                 kind="ExternalInput")
        D_in[d] = nc.dram_tensor(f"D_{d}", [CH, 1], dt.float32,
                                 kind="ExternalInput")
    ident = nc.dram_tensor("ident", [P, P], dt.float32, kind="ExternalInput")
    out = nc.dram_tensor("out", [L // num_cores, DM], dt.float32,
                         kind="ExternalOutput")

    NKB = DM // P  # 8
    NTT = L // P   # 16
    WPAD = L + 2 * PAD
    TW = 2 * L     # pb-fused tile width (pb0 cols 0:L, pb1 cols L:TW)

    with tile.TileContext(nc) as tc:
        ctx = ExitStack()
        with ctx:
            dram = ctx.enter_context(tc.tile_pool(name="dram", bufs=1, space="DRAM"))
            consts = ctx.enter_context(tc.tile_pool(name="consts", bufs=1))
            psum_mm = ctx.enter_context(
                tc.tile_pool(name="psum_mm", bufs=3, space="PSUM"))

            ident_sb = consts.tile([P, P], dt.float32)
            nc.sync.dma_start(ident_sb[:], ident[:])

            # ---------------- persistent pools (live through the scan phase)
            gz_pool = ctx.enter_context(tc.tile_pool(name="gzp", bufs=1))
            gz = {}
            for d in ("f", "r"):
                gz[d] = gz_pool.tile([P, TW], dt.bfloat16, name=f"gz{d}",
                                     tag=f"gz{d}")
            yc_pool = ctx.enter_context(tc.tile_pool(name="ycp", bufs=2))
            oev_pool = ctx.enter_context(tc.tile_pool(name="oevp", bufs=2))
            dt_pool = ctx.enter_context(tc.tile_pool(name="dtp", bufs=1))
            dtx_pool = ctx.enter_context(tc.tile_pool(name="dtxp", bufs=2))
            xc_pool = ctx.enter_context(tc.tile_pool(name="xcp", bufs=2))
            et_pool = ctx.enter_context(tc.tile_pool(name="etp", bufs=4))
            xdbl_pool = ctx.enter_context(tc.tile_pool(name="xdblp", bufs=1))

            xdbl_part = {}
            xdbl_sum = {}
            for d in ("f", "r"):
                xdbl_part[d] = dram.tile([NXP, L], dt.bfloat16, name=f"xp{d}",
                                         tag=f"xp{d}")
                xdbl_sum[d] = dram.tile([NXP, L], dt.bfloat16,
                                        addr_space="Shared", name=f"xs{d}",
                                        tag=f"xs{d}")
            pout = dram.tile([L, DM], dt.float32)
            pout_rs = dram.tile([L // num_cores, DM], dt.float32)

            # stage-limited pools
            ctxB = ExitStack()
            xpad_pool = ctxB.enter_context(tc.tile_pool(name="xpadp", bufs=1))
            cacc_pool = ctxB.enter_context(tc.tile_pool(name="caccp", bufs=2))
            xev_pool = ctxB.enter_context(tc.tile_pool(name="xevp", bufs=2))
            ctxA = ExitStack()
            hT_pool = ctxA.enter_context(tc.tile_pool(name="hTp", bufs=1))
            hnat_pool = ctxA.enter_context(tc.tile_pool(name="hnatp", bufs=4))
            w_in_pool = ctxA.enter_context(tc.tile_pool(name="winp", bufs=1))

            # ------------------------------------------- stage 1: hT = hidden^T
            psum_tp = ctxA.enter_context(
                tc.tile_pool(name="psum_tp", bufs=3, space="PSUM"))
            hT = {(k, q): hT_pool.tile([P, 4 * P], dt.bfloat16,
                                       name=f"hT{k}_{q}", tag=f"hT{k}_{q}")
                  for k in range(NKB) for q in range(NTT // 4)}
            for q in range(NTT // 4):
                hn = []
                for j in range(4):
                    t = hnat_pool.tile([P, DM], dt.float32, name="hnat", tag="hnat")
                    nc.sync.dma_start(
                        t[:], hidden[(q * 4 + j) * P:(q * 4 + j + 1) * P, :])
                    hn.append(t)
                for kb in range(NKB):
                    pt = psum_tp.tile([P, 4 * P], dt.float32, name="tp", tag="tp")
                    for j in range(4):
                        nc.tensor.transpose(
                            pt[:, j * P:(j + 1) * P],
                            hn[j][:, kb * P:(kb + 1) * P],
                            ident_sb[:],
                        )
                    nc.scalar.copy(hT[kb, q][:], pt[:])

            # ------------------------------------------- stage 2: in_proj x-half
            w_in_sb = [w_in_pool.tile([P, 2 * CH], dt.bfloat16, name=f"win{k}",
                                      tag=f"win{k}") for k in range(NKB)]
            for k in range(NKB):
                nc.sync.dma_start(w_in_sb[k][:], w_inT[k * P:(k + 1) * P, :])

            # const loads (after the latency-critical hidden/w_in DMAs)
            conv_w_sb = {}
            conv_b_sb = {}
            dt_b_sb = {}
            A_sb = {}
            D_sb = {}
            for d in ("f", "r"):
                for pb in range(NPT):
                    ps = slice(pb * P, (pb + 1) * P)
                    for nm, store, srct, shape in (
                        ("cw", conv_w_sb, conv_w, [P, KCONV]),
                        ("cb", conv_b_sb, conv_b, [P, 1]),
                        ("db", dt_b_sb, dt_b, [P, 1]),
                        ("A", A_sb, A_in, [P, NST]),
                        ("Dc", D_sb, D_in, [P, 1]),
                    ):
                        t = consts.tile(shape, dt.float32, name=f"{nm}{d}{pb}",
                                        tag=f"{nm}{d}{pb}")
                        nc.sync.dma_start(t[:], srct[d][ps, :])
                        store[d, pb] = t
            w_dt_sb = {}
            for d in ("f", "r"):
                w_dt_sb[d] = consts.tile([RK, CH], dt.bfloat16, name=f"wdt{d}",
                                         tag=f"wdt{d}")
                nc.sync.dma_start(w_dt_sb[d][:], w_dtT[d][:])
            w_x_sb = {}
            for d in ("f", "r"):
                for pb in range(NPT):
                    t = consts.tile([P, NXP], dt.bfloat16, name=f"wx{d}{pb}",
                                    tag=f"wx{d}{pb}")
                    nc.sync.dma_start(t[:], w_xT[d][pb * P:(pb + 1) * P, :])
                    w_x_sb[d, pb] = t
            w_out_sb = []
            for pb in range(NPT):
                t = consts.tile([P, DM], dt.bfloat16, name=f"wo{pb}", tag=f"wo{pb}")
                nc.sync.dma_start(t[:], w_outT[pb * P:(pb + 1) * P, :])
                w_out_sb.append(t)

            xpad = [xpad_pool.tile([P, WPAD], dt.bfloat16, name=f"xpad{pb}",
                                   tag=f"xpad{pb}") for pb in range(NPT)]
            for pb in range(NPT):
                nc.vector.memset(xpad[pb][:, 0:PAD], 0.0)
                nc.vector.memset(xpad[pb][:, PAD + L:WPAD], 0.0)

            for fb in range(NFB):
                for mb in range(NPT):
                    pm = psum_mm.tile([P, FB], dt.float32, name="mm", tag="mm")
                    for k in range(NKB):
                        nc.tensor.matmul(
                            pm[:],
                            w_in_sb[k][:, mb * P:(mb + 1) * P],
                            hT[k, fb][:],
                            start=(k == 0),
                            stop=(k == NKB - 1),
                        )
                    nc.scalar.copy(
                        xpad[mb][:, PAD + fb * FB: PAD + (fb + 1) * FB], pm[:])

            xc = {}

            def conv_block(d):
                """causal depthwise conv + silu -> fused xc[d] [P, TW]."""
                xc[d] = xc_pool.tile([P, TW], dt.bfloat16, name=f"xc{d}",
                                     tag="xc")
                for pb in range(NPT):
                    cw = conv_w_sb[d, pb]
                    cb = conv_b_sb[d, pb]
                    if d == "f":
                        taps = [xpad[pb][:, k:k + L] for k in range(KCONV)]
                    else:
                        taps = [xpad[pb][:, 2 * PAD - k: 2 * PAD - k + L][:, ::-1]
                                for k in range(KCONV)]
                    acc = cacc_pool.tile([P, L], dt.bfloat16, name="cacc", tag="cacc")
                    nc.scalar.activation(acc[:], taps[0], AF.Identity,
                                         bias=cb[:, 0:1], scale=cw[:, 0:1])
                    for k in range(1, KCONV):
                        acc2 = cacc_pool.tile([P, L], dt.bfloat16, name="cacc",
                                              tag="cacc")
                        nc.vector.scalar_tensor_tensor(
                            acc2[:], taps[k], cw[:, k:k + 1], acc[:],
                            OP.mult, OP.add)
                        acc = acc2
                    nc.scalar.activation(
                        xc[d][:, pb * L:(pb + 1) * L], acc[:], AF.Silu)

            def xproj_block(d):
                for pb in range(NPT):
                    for fb in range(NFB):
                        pm = psum_mm.tile([NXP, FB], dt.float32, name="mmx",
                                          tag="mm")
                        nc.tensor.matmul(
                            pm[:],
                            w_x_sb[d, 0][:],
                            xc[d][:, fb * FB:(fb + 1) * FB] if pb == 0 else
                            xc[d][:, L + fb * FB:L + (fb + 1) * FB],
                            start=True, stop=False,
                            skip_group_check=True,
                        ) if False else None
                # standard: accumulate the two pb contributions per fb chunk
                for fb in range(NFB):
                    pm = psum_mm.tile([NXP, FB], dt.float32, name="mmx", tag="mm")
                    for pb in range(NPT):
                        nc.tensor.matmul(
                            pm[:],
                            w_x_sb[d, pb][:],
                            xc[d][:, pb * L + fb * FB: pb * L + (fb + 1) * FB],
                            start=(pb == 0),
                            stop=(pb == NPT - 1),
                        )
                    xev = xev_pool.tile([NXP, FB], dt.bfloat16, name="xev",
                                        tag="xev")
                    nc.scalar.copy(xev[:], pm[:])
                    nc.sync.dma_start(
                        xdbl_part[d][:, fb * FB:(fb + 1) * FB], xev[:])

            def allreduce(d):
                nc.gpsimd.collective_compute(
                    "AllReduce",
                    OP.add,
                    replica_groups=[list(range(num_cores))],
                    ins=[xdbl_part[d][:].opt()],
                    outs=[xdbl_sum[d][:].opt()],
                )

            conv_block("f")
            xproj_block("f")
            allreduce("f")
            conv_block("r")
            xproj_block("r")
            allreduce("r")

            # ------------------- z half of in_proj + silu gating (overlaps AR)
            for mb in range(NPT, 2 * NPT):
                for fb in range(NFB):
                    pm = psum_mm.tile([P, FB], dt.float32, name="mm", tag="mm")
                    for k in range(NKB):
                        nc.tensor.matmul(
                            pm[:],
                            w_in_sb[k][:, mb * P:(mb + 1) * P],
                            hT[k, fb][:],
                            start=(k == 0),
                            stop=(k == NKB - 1),
                        )
                    pb = mb - NPT
                    nc.scalar.activation(
                        gz["f"][:, pb * L + fb * FB: pb * L + (fb + 1) * FB],
                        pm[:], AF.Silu)
                    grev = gz["r"][:, pb * L:(pb + 1) * L][:, ::-1]
                    nc.scalar.activation(
                        grev[:, fb * FB:(fb + 1) * FB], pm[:], AF.Silu)

            ctxA.close()

            dt_sb = {}
            dtx = {}
            xdbl = {}
            y = {}
            accv = {}

            def dt_block(d, xc_tile):
                """x_dbl dt-rows -> softplus dt, dtx, chain seed (D*x)."""
                xdbl[d] = xdbl_pool.tile([RK, L], dt.bfloat16, name="xdbl",
                                         tag="xdbl")
                nc.sync.dma_start(xdbl[d][:], xdbl_sum[d][0:RK, :])
                dt_sb[d] = dt_pool.tile([P, TW], dt.bfloat16, name="dtt",
                                        tag=f"dtt{d}")
                for pb in range(NPT):
                    ets = {}
                    for fb in range(NFB):
                        pm = psum_mm.tile([P, FB], dt.float32, name="mm",
                                          tag="mm")
                        nc.tensor.matmul(
                            pm[:],
                            w_dt_sb[d][:, pb * P:(pb + 1) * P],
                            xdbl[d][:, fb * FB:(fb + 1) * FB],
                            start=True, stop=True)
                        et = et_pool.tile([P, FB], dt.bfloat16, name="etmp",
                                          tag="etmp")
                        nc.scalar.activation(
                            et[:], pm[:], AF.Exp, bias=dt_b_sb[d, pb][:, 0:1])
                        ets[fb] = et
                    for fb in range(NFB):
                        nc.scalar.activation(
                            dt_sb[d][:, pb * L + fb * FB: pb * L + (fb + 1) * FB],
                            ets[fb][:], AF.Ln, bias=1.0)
                tx = dtx_pool.tile([P, TW], dt.bfloat16, name="dtx", tag="dtx")
                nc.vector.tensor_mul(tx[:], dt_sb[d][:], xc_tile[:])
                dtx[d] = tx
                # poison the pb seam so the fused scan restarts at col L
                nc.vector.memset(dt_sb[d][:, L:L + 1], 1.0e9)

            def seed_chain(d, xc_tile):
                """accv[d] = D*x (the reduction chain grows from this seed)."""
                s = accv_pool.tile([P, TW], dt.bfloat16, name="seed", tag="acc")
                for pb in range(NPT):
                    nc.vector.tensor_scalar_mul(
                        s[:, pb * L:(pb + 1) * L],
                        xc_tile[:, pb * L:(pb + 1) * L],
                        D_sb[d, pb][:, 0:1])
                accv[d] = s

            def scan_iter(d, n):
                rb = xdbl_sum[d][RK + n: RK + n + 1, :]
                rc = xdbl_sum[d][RK + NST + n: RK + NST + n + 1, :]
                bb = bbc_pool.tile([P, TW], dt.bfloat16, name="bbc", tag="bbc")
                nc.sync.dma_start(
                    bb[:], bass.AP(rb.tensor, rb.offset, [[0, P], [0, 2], [1, L]]))
                cbt = cbc_pool.tile([P, TW], dt.bfloat16, name="cbc", tag="cbc")
                nc.sync.dma_start(
                    cbt[:], bass.AP(rc.tensor, rc.offset, [[0, P], [0, 2], [1, L]]))
                da = da_pool.tile([P, TW], dt.float16, name="da", tag="da")
                for pb in range(NPT):
                    nc.scalar.activation(
                        da[:, pb * L:(pb + 1) * L],
                        dt_sb[d][:, pb * L:(pb + 1) * L], AF.Exp,
                        scale=A_sb[d, pb][:, n:n + 1])
                dbx = dbx_pool.tile([P, TW], dt.bfloat16, name="dbx", tag="dbx")
                nc.gpsimd.tensor_tensor(dbx[:], dtx[d][:], bb[:], OP.mult)
                h = h_pool.tile([P, TW], dt.bfloat16, name="h", tag="h")
                nc.vector.tensor_tensor_scan(
                    h[:], da[:], dbx[:], 0.0, OP.mult, OP.add)
                hc = hc_pool.tile([P, TW], dt.bfloat16, name="hc", tag="hc")
                nc.vector.tensor_mul(hc[:], h[:], cbt[:])
                s = accv_pool.tile([P, TW], dt.bfloat16, name="acc", tag="acc")
                nc.vector.tensor_add(s[:], accv[d][:], hc[:])
                accv[d] = s

            def collapse(d):
                yt = dtx_pool.tile([P, TW], dt.bfloat16, name="y", tag="dtx")
                nc.vector.tensor_mul(yt[:], accv[d][:], gz[d][:])
                y[d] = yt

            dt_block("f", xc["f"])
            ctxB.close()

            # ------------------------------------------- scan pools
            bbc_pool = ctx.enter_context(tc.tile_pool(name="bbcp", bufs=2))
            cbc_pool = ctx.enter_context(tc.tile_pool(name="cbcp", bufs=2))
            da_pool = ctx.enter_context(tc.tile_pool(name="dap", bufs=3))
            dbx_pool = ctx.enter_context(tc.tile_pool(name="dbxp", bufs=3))
            h_pool = ctx.enter_context(tc.tile_pool(name="hp", bufs=1))
            hc_pool = ctx.enter_context(tc.tile_pool(name="hcp", bufs=1))
            accv_pool = ctx.enter_context(tc.tile_pool(name="accvp", bufs=2))

            # (seed uses accv_pool which must exist before dt_block refs run;
            # Python late-binding makes this fine because dt_block("f") above
            # only used pools already created. seed_chain for "f" runs here.)
            seed_chain("f", xc["f"])

            if skip_scan:
                for d in ("f", "r"):
                    if d == "r":
                        dt_block("r", xc["r"])
                    yt = dtx_pool.tile([P, TW], dt.bfloat16, name="y", tag="dtx")
                    nc.vector.tensor_mul(yt[:], dtx[d][:], gz[d][:])
                    y[d] = yt
            else:
                for n in range(6):
                    scan_iter("f", n)
                # dt_block("r") lands here (AR_r done by now)
                dt_block("r", xc["r"])
                for n in range(6, NST):
                    scan_iter("f", n)
                collapse("f")
                seed_chain("r", xc["r"])
                for n in range(NST):
                    scan_iter("r", n)
                collapse("r")

            # ------------------------------------------- out_proj + RS
            for rs in range(NRS):
                for tbo in range(RSROWS // P):
                    tb = rs * (RSROWS // P) + tbo
                    ycb = {}
                    for pb in range(NPT):
                        yct = yc_pool.tile([P, P], dt.bfloat16, name="ycb",
                                           tag="ycb")
                        nc.vector.tensor_add(
                            yct[:],
                            y["f"][:, pb * L + tb * P: pb * L + (tb + 1) * P],
                            y["r"][:, pb * L:(pb + 1) * L][:, ::-1]
                            [:, tb * P:(tb + 1) * P])
                        ycb[pb] = yct
                    for fb in range(DM // FB):
                        pm = psum_mm.tile([P, FB], dt.float32, name="mm",
                                          tag="mm")
                        for pb in range(NPT):
                            nc.tensor.matmul(
                                pm[:],
                                ycb[pb][:],
                                w_out_sb[pb][:, fb * FB:(fb + 1) * FB],
                                start=(pb == 0),
                                stop=(pb == NPT - 1),
                            )
                        oev = oev_pool.tile([P, FB], dt.float32, name="oev",
                                            tag="oev")
                        nc.scalar.copy(oev[:], pm[:])
                        nc.sync.dma_start(
                            pout[tb * P:(tb + 1) * P, fb * FB:(fb + 1) * FB],
                            oev[:])
                nc.gpsimd.collective_compute(
                    "ReduceScatter",
                    OP.add,
                    replica_groups=[list(range(num_cores))],
                    ins=[pout[rs * RSROWS:(rs + 1) * RSROWS, :].opt()],
                    outs=[pout_rs[rs * (RSROWS // num_cores):
                                  (rs + 1) * (RSROWS // num_cores), :].opt()],
                )
                nc.sync.dma_start(
                    out[rs * (RSROWS // num_cores):
                        (rs + 1) * (RSROWS // num_cores), :],
                    pout_rs[rs * (RSROWS // num_cores):
                            (rs + 1) * (RSROWS // num_cores), :])

    return nc


# ---------------------------------------------------------------- host side
def _make_in_maps(inputs):
    """Slice/transpose the full inputs into per-core input dicts."""
    h = np.ascontiguousarray(np.asarray(inputs["hidden_states"],
                                        dtype=np.float32).reshape(L, DM))
    w_in = np.asarray(inputs["in_proj_w"], dtype=np.float32)     # (2DI, DM)
    w_out = np.asarray(inputs["out_proj_w"], dtype=np.float32)   # (DM, DI)
    ident = np.eye(P, dtype=np.float32)

    in_maps = []
    for c in range(NCORES):
        sl = slice(c * CH, (c + 1) * CH)
        m = {"hidden": h, "ident": ident}
        w_slice = np.concatenate(
            [w_in[sl, :], w_in[DI + c * CH: DI + (c + 1) * CH, :]], axis=0)
        m["w_inT"] = np.ascontiguousarray(
            w_slice.T).astype(ml_dtypes.bfloat16)                 # (DM, 2CH)
        m["w_outT"] = np.ascontiguousarray(
            w_out[:, sl].T).astype(ml_dtypes.bfloat16)            # (CH, DM)
        for d, tag in (("f", "_f"), ("r", "_r")):
            w_x = np.asarray(inputs[f"x_proj_w{tag}"], dtype=np.float32)
            m[f"w_xT_{d}"] = np.ascontiguousarray(
                w_x[:, sl].T).astype(ml_dtypes.bfloat16)          # (CH, 96)
            w_dt = np.asarray(inputs[f"dt_proj_w{tag}"], dtype=np.float32)
            m[f"w_dtT_{d}"] = np.ascontiguousarray(
                w_dt[sl, :].T).astype(ml_dtypes.bfloat16)         # (RK, CH)
            m[f"conv_w_{d}"] = np.ascontiguousarray(
                np.asarray(inputs[f"conv_w{tag}"], dtype=np.float32)[sl, :])
            m[f"conv_b_{d}"] = np.ascontiguousarray(
                np.asarray(inputs[f"conv_b{tag}"], dtype=np.float32)[sl, None])
            m[f"dt_b_{d}"] = np.ascontiguousarray(
                np.asarray(inputs[f"dt_proj_b{tag}"], dtype=np.float32)[sl, None])
            m[f"A_{d}"] = np.ascontiguousarray(
                -np.exp(np.asarray(inputs[f"A_log{tag}"], dtype=np.float32)[sl, :]))
            m[f"D_{d}"] = np.ascontiguousarray(
                np.asarray(inputs[f"D{tag}"], dtype=np.float32)[sl, None])
        in_maps.append(m)
    return in_maps


_CACHED = {}


def _install_ntff_hook_shim():
    """The agent image's antenv lacks axon_hooks; provide it and register
    the ctypes-based NTFF profile hook from trn_agent_boot."""
    import types
    try:
        import antenv.axon_hooks  # noqa: F401
        return
    except ImportError:
        pass
    import antenv
    mod = types.ModuleType("antenv.axon_hooks")
    _state = {"h": None}
    mod.get_axon_ntff_profile_hook = lambda: _state["h"]
    mod.set_axon_ntff_profile_hook = lambda h: _state.__setitem__("h", h)
    sys.modules["antenv.axon_hooks"] = mod
    antenv.axon_hooks = mod
    try:
        from trn_agent_boot.trn_boot import _ntff_profile_via_ctypes
        hook = _ntff_profile_via_ctypes("/opt/axon/libaxon_pjrt.so")
        if hook is not None:
            mod.set_axon_ntff_profile_hook(hook)
    except Exception:
        pass


def _install_hook_err_capture():
    """Wrap the neuronx_cc hook so compile errors land in hook_err.log
    instead of being swallowed by the PJRT boundary."""
    import traceback
    import concourse.bass2jax as b2j
    if getattr(b2j, "_err_capture_installed", False):
        return
    orig = b2j.neuronx_cc_hook

    def wrapped(*a):
        try:
            return orig(*a)
        except Exception:
            with open("/tmp/hook_err.log", "w") as f:
                f.write(traceback.format_exc())
            raise

    b2j.neuronx_cc_hook = wrapped
    b2j._err_capture_installed = True


def kernel(**inputs):
    from concourse.bass_utils import run_bass_kernel_spmd

    _install_ntff_hook_shim()
    _install_hook_err_capture()

    if "nc" not in _CACHED:
        from concourse.bass_interp import get_hw_module
        nc = build_program(
            skip_scan=bool(int(os.environ.get("KERNEL_SKIP_SCAN", "0"))))
        nc.finalize()  # bacc: register allocation, library/ACT-table loads
        nc.m = get_hw_module(nc.m)  # strip sim-only callback instructions
        _CACHED["nc"] = nc
    nc = _CACHED["nc"]

    in_maps = _make_in_maps(inputs)
    res = run_bass_kernel_spmd(
        nc, in_maps, core_ids=list(range(NCORES)),
        trace=bool(int(os.environ.get("KERNEL_TRACE", "0"))),
    )
    _CACHED["last_result"] = res
    # Chunked ReduceScatter permutes row ownership: core c's out rows are
    # [rs*RSROWS + c*(RSROWS/8) : +RSROWS/8) for each rs chunk.
    rows = RSROWS // NCORES
    full = np.empty((L, DM), dtype=np.float32)
    for c in range(NCORES):
        o = res.results[c]["out"]
        for rs in range(NRS):
            full[rs * RSROWS + c * rows: rs * RSROWS + (c + 1) * rows, :] = \
                o[rs * rows:(rs + 1) * rows, :]
    return full.reshape(1, L, DM)


if __name__ == "__main__":
    nc = build_program()
    try:
        n = sum(len(bb.instructions) for bb in nc.main_func.blocks)
    except Exception:
        n = "?"
    print("build ok; instructions:", n)
